# revision 3
# baseline (speedup 1.0000x reference)
"""Causal multi-head attention (B=4, S=2048, D=1024, H=16) on 8 NeuronCores.

Sharding: core c = (batch b = c//2, head-group hg = c%2). Each core computes
8 heads of one batch: QKV projection (fp32r matmuls), causal flash-style
attention (bf16 matmuls, exp-without-max softmax with the ones-column
denominator trick), and a row-parallel out-projection partial (fp32r).
Host sums the two head-group partials per batch and transposes.

All on-device layouts are feature-major ([feature, token]) except v, which
is produced token-major so attn@v needs no transposes.
"""
import numpy as np
from contextlib import ExitStack

import ml_dtypes

B, S, D, H = 4, 2048, 1024, 16
HD = 64            # head dim
HPC = 8            # heads per core
F = HPC * HD       # 512 features per head-group
QT = 512           # q tile (free dim)
KT = 128           # k tile (partition dim)
NQI = S // QT      # 4
NKT = S // KT      # 16
NDK = D // 128     # 8 contraction tiles for projections
SCALE = HD ** -0.5

_CACHE = {}


def _build():
    import concourse.bacc as bacc
    import concourse.tile as tile
    import concourse.mybir as mybir

    f32 = mybir.dt.float32
    f32r = mybir.dt.float32r
    bf16 = mybir.dt.bfloat16
    EXP = mybir.ActivationFunctionType.Exp
    IDENT = mybir.ActivationFunctionType.Identity

    nc = bacc.Bacc("TRN2", target_bir_lowering=False, debug=False)
    xT = nc.dram_tensor("xT", [D, S], f32r, kind="ExternalInput").ap()
    w_sl = nc.dram_tensor("w_sl", [D, 3 * F], f32r, kind="ExternalInput").ap()
    wo_sl = nc.dram_tensor("wo_sl", [F, D], f32r, kind="ExternalInput").ap()
    bias_t = nc.dram_tensor("bias_t", [128, 8], f32, kind="ExternalInput").ap()
    mask = nc.dram_tensor("mask", [128, 128], bf16, kind="ExternalInput").ap()
    out = nc.dram_tensor("out", [D, S], f32, kind="ExternalOutput").ap()

    with tile.TileContext(nc) as tc:
        with ExitStack() as ctx:
            # ---- persistent pools (outputs of QKV + constants) ----
            misc = ctx.enter_context(tc.tile_pool(name="misc", bufs=1))
            mask_sb = misc.tile([128, 128], bf16, name="mask_sb", tag="mask")
            nc.sync.dma_start(mask_sb[:], mask)
            bias_sb = misc.tile([128, 8], f32, name="bias_sb", tag="bias")
            nc.sync.dma_start(bias_sb[:], bias_t)

            pqk = ctx.enter_context(tc.tile_pool(name="pqk", bufs=1))
            pv = ctx.enter_context(tc.tile_pool(name="pv", bufs=1))

            q_sb = [pqk.tile([128, S], bf16, name=f"q{g}", tag=f"q{g}")
                    for g in range(4)]
            k_sb = [pqk.tile([128, S], bf16, name=f"k{g}", tag=f"k{g}")
                    for g in range(4)]
            v_sb = [pv.tile([128, HPC * (HD + 1)], bf16, name=f"v{t}", tag=f"v{t}")
                    for t in range(NKT)]

            # ---- phase A: QKV projection (fp32r) ----
            with tc.tile_pool(name="xw", bufs=1) as xw, \
                 tc.tile_pool(name="psA", bufs=4, space="PSUM") as psA:
                x_t = []
                w_t = []
                for kk in range(NDK):
                    xt = xw.tile([128, S], f32r, name=f"x{kk}", tag=f"x{kk}")
                    nc.sync.dma_start(xt[:], xT[kk * 128:(kk + 1) * 128, :])
                    x_t.append(xt)
                    wt = xw.tile([128, 3 * F], f32r, name=f"w{kk}", tag=f"w{kk}")
                    nc.sync.dma_start(wt[:], w_sl[kk * 128:(kk + 1) * 128, :])
                    w_t.append(wt)

                # q, k: feature-major [feat, tok]; w stationary, x moving
                for part, dest in ((0, q_sb), (1, k_sb)):
                    for g in range(4):
                        fcol = part * F + g * 128
                        for tg in range(NQI):
                            ps = psA.tile([128, QT], f32, name=f"pqk{part}{g}{tg}",
                                          tag="qkv")
                            for kk in range(NDK):
                                nc.tensor.matmul(
                                    ps[:],
                                    w_t[kk][:, fcol:fcol + 128],
                                    x_t[kk][:, tg * QT:(tg + 1) * QT],
                                    start=(kk == 0), stop=(kk == NDK - 1))
                            nc.vector.tensor_copy(
                                dest[g][:, tg * QT:(tg + 1) * QT], ps[:])

                # v: token-major [tok, feat]; x stationary, w moving
                for tt in range(NKT):
                    ps = psA.tile([128, F], f32, name=f"pv{tt}", tag="qkv")
                    for kk in range(NDK):
                        nc.tensor.matmul(
                            ps[:],
                            x_t[kk][:, tt * 128:(tt + 1) * 128],
                            w_t[kk][:, 2 * F:3 * F],
                            start=(kk == 0), stop=(kk == NDK - 1))
                    # strided copy into per-head [64 v | 1 ones] layout
                    vv = v_sb[tt].rearrange("p (h c) -> p h c", h=HPC)
                    pp = ps.rearrange("p (h c) -> p h c", h=HPC)
                    nc.vector.tensor_copy(vv[:, :, 0:HD], pp[:])
                    nc.vector.memset(vv[:, :, HD:HD + 1], 1.0)

            # ---- phase B: causal attention, head pairs packed in PE rows ----
            patt = ctx.enter_context(tc.tile_pool(name="patt", bufs=1))
            att_m = {}
            pP = ctx.enter_context(tc.tile_pool(name="pP", bufs=6))
            pr = ctx.enter_context(tc.tile_pool(name="pr", bufs=4))
            ptmp = ctx.enter_context(tc.tile_pool(name="ptmp", bufs=3))

            with tc.tile_pool(name="psB", bufs=4, space="PSUM") as psB:
                for pg in range(4):
                    for qi in range(NQI):
                        nkt = 4 * qi + 4
                        ao_e = psB.tile([HD + 1, QT], f32,
                                        name=f"aoe{pg}{qi}", tag="ao")
                        ao_o = psB.tile([HD + 1, QT], f32,
                                        name=f"aoo{pg}{qi}", tag="ao")
                        for kt in range(nkt):
                            d = kt - 4 * qi
                            n0 = 0 if d < 0 else 128 * d
                            qs = qi * QT
                            kcol = slice(kt * 128, kt * 128 + 128)
                            sc_e = psB.tile([128, QT], f32,
                                            name=f"sce{pg}{qi}{kt}", tag="sc")
                            sc_o = psB.tile([128, QT], f32,
                                            name=f"sco{pg}{qi}{kt}", tag="sc")
                            nc.tensor.matmul(
                                sc_e[:, n0:QT], k_sb[pg][0:64, kcol],
                                q_sb[pg][0:64, qs + n0:qs + QT],
                                start=True, stop=True)
                            nc.tensor.matmul(
                                sc_o[:, n0:QT], k_sb[pg][64:128, kcol],
                                q_sb[pg][64:128, qs + n0:qs + QT],
                                start=True, stop=True)
                            p_e = pP.tile([128, QT], bf16,
                                          name=f"pe{pg}{qi}{kt}", tag="P")
                            p_o = pP.tile([128, QT], bf16,
                                          name=f"po{pg}{qi}{kt}", tag="P")
                            nc.scalar.activation(p_e[:, n0:QT], sc_e[:, n0:QT],
                                                 EXP, scale=SCALE)
                            nc.scalar.activation(p_o[:, n0:QT], sc_o[:, n0:QT],
                                                 EXP, scale=SCALE)
                            if d >= 0:
                                nc.vector.tensor_mul(
                                    p_e[:, n0:n0 + 128], p_e[:, n0:n0 + 128],
                                    mask_sb[:])
                                nc.vector.tensor_mul(
                                    p_o[:, n0:n0 + 128], p_o[:, n0:n0 + 128],
                                    mask_sb[:])
                            st = (kt == 0)
                            sp = (kt == nkt - 1)
                            he = 2 * pg
                            ho = 2 * pg + 1
                            C = HD + 1
                            nc.tensor.matmul(
                                ao_e[:, n0:QT], v_sb[kt][:, he * C:(he + 1) * C],
                                p_e[:, n0:QT], start=st, stop=sp)
                            nc.tensor.matmul(
                                ao_o[:, n0:QT], v_sb[kt][:, ho * C:(ho + 1) * C],
                                p_o[:, n0:QT], start=st, stop=sp)

                        # normalize: r = 1/rowsum (row HD); gpsimd broadcasts
                        # partition 0 -> 0..63; DVE writes Q2/Q3 directly
                        am = patt.tile([128, QT], f32r,
                                       name=f"am{pg}{qi}", tag=f"am{pg}{qi}")
                        att_m[(pg, qi)] = am
                        rr_e = ptmp.tile([1, QT], f32, name=f"rre{pg}{qi}",
                                         tag="rr")
                        nc.vector.reciprocal(rr_e[:], ao_e[HD:HD + 1, :])
                        r_e = pr.tile([HD, QT], f32, name=f"re{pg}{qi}", tag="r")
                        nc.gpsimd.partition_broadcast(r_e[:], rr_e[:],
                                                      channels=HD)
                        nc.vector.tensor_mul(am[0:64, :], ao_e[0:HD, :],
                                             r_e[:])
                        rr_o = ptmp.tile([1, QT], f32, name=f"rro{pg}{qi}",
                                         tag="rr")
                        nc.vector.reciprocal(rr_o[:], ao_o[HD:HD + 1, :])
                        r_o = pr.tile([HD, QT], f32, name=f"ro{pg}{qi}", tag="r")
                        nc.gpsimd.partition_broadcast(r_o[:], rr_o[:],
                                                      channels=HD)
                        nc.vector.tensor_mul(am[64:128, :], ao_o[0:HD, :],
                                             r_o[:])

            # ---- phase C: out projection (fp32r), head pairs packed ----
            with tc.tile_pool(name="pwo", bufs=1) as pwo, \
                 tc.tile_pool(name="pstg", bufs=3) as pstg, \
                 tc.tile_pool(name="psC", bufs=2, space="PSUM") as psC:
                wo_t = []
                for g in range(4):
                    wt = pwo.tile([128, D], f32r, name=f"wo{g}", tag=f"wo{g}")
                    nc.sync.dma_start(wt[:], wo_sl[g * 128:(g + 1) * 128, :])
                    wo_t.append(wt)

                for dt in range(8):
                    dcol = slice(dt * 128, dt * 128 + 128)
                    for qi in range(NQI):
                        ps_e = psC.tile([128, QT], f32,
                                        name=f"oe{dt}{qi}", tag="ope")
                        ps_o = psC.tile([128, QT], f32,
                                        name=f"oo{dt}{qi}", tag="opo")
                        for pg in range(4):
                            am = att_m[(pg, qi)]
                            nc.tensor.matmul(
                                ps_e[:], wo_t[pg][0:64, dcol], am[0:64, :],
                                start=(pg == 0), stop=(pg == 3))
                            nc.tensor.matmul(
                                ps_o[:], wo_t[pg][64:128, dcol], am[64:128, :],
                                start=(pg == 0), stop=(pg == 3))
                        s1 = pstg.tile([128, QT], f32, name=f"s1{dt}{qi}",
                                       tag="s1")
                        nc.scalar.activation(s1[:], ps_e[:], IDENT,
                                             bias=bias_sb[:, dt:dt + 1])
                        s2 = pstg.tile([128, QT], f32, name=f"s2{dt}{qi}",
                                       tag="s2")
                        nc.vector.tensor_add(s2[:], ps_o[:], s1[:])
                        nc.sync.dma_start(
                            out[dt * 128:(dt + 1) * 128,
                                qi * QT:(qi + 1) * QT], s2[:])

    nc.compile()
    return nc


def _get_nc():
    if "nc" not in _CACHE:
        _CACHE["nc"] = _build()
    return _CACHE["nc"]


def _prep_inputs(x, w_qkv, w_out, b_out):
    """Build the 8 per-core input maps."""
    x = np.asarray(x, dtype=np.float32)
    w_qkv = np.asarray(w_qkv, dtype=np.float32)
    w_out = np.asarray(w_out, dtype=np.float32)
    b_out = np.asarray(b_out, dtype=np.float32)

    mask = np.triu(np.ones((128, 128), dtype=np.float32)).astype(
        ml_dtypes.bfloat16)
    zeros_bias = np.zeros((128, 8), dtype=np.float32)
    bias_t = np.ascontiguousarray(b_out.reshape(8, 128).T)

    in_maps = []
    for c in range(8):
        b, hg = c // 2, c % 2
        cols = hg * F
        w_cat = np.concatenate([
            w_qkv[:, cols:cols + F],
            w_qkv[:, D + cols:D + cols + F],
            w_qkv[:, 2 * D + cols:2 * D + cols + F],
        ], axis=1)
        in_maps.append({
            "xT": np.ascontiguousarray(x[b].T),
            "w_sl": np.ascontiguousarray(w_cat),
            "wo_sl": np.ascontiguousarray(w_out[cols:cols + F, :]),
            "bias_t": bias_t if hg == 0 else zeros_bias,
            "mask": mask,
        })
    return in_maps


def _run(inputs, trace=False):
    from concourse.bass_utils import run_bass_kernel_spmd

    nc = _get_nc()
    in_maps = _prep_inputs(**inputs)
    res = run_bass_kernel_spmd(nc, in_maps, core_ids=list(range(8)),
                               trace=trace)
    outs = []
    for b in range(B):
        o = res.results[2 * b]["out"] + res.results[2 * b + 1]["out"]
        outs.append(o.T)
    full = np.stack(outs).astype(np.float32)
    return full, res


def kernel(x, w_qkv, w_out, b_out):
    full, _ = _run({"x": x, "w_qkv": w_qkv, "w_out": w_out, "b_out": b_out})
    return full


# revision 7
# speedup vs baseline: 1.3113x; 1.3113x over previous
"""Causal multi-head attention (B=4, S=2048, D=1024, H=16) on 8 NeuronCores.

Sharding: core c = (batch b = c//2, head-group hg = c%2). Each core computes
8 heads of one batch: QKV projection (fp32r matmuls), causal flash-style
attention (bf16 matmuls, exp-without-max softmax with the ones-column
denominator trick), and a row-parallel out-projection partial (fp32r).
Host sums the two head-group partials per batch and transposes.

All on-device layouts are feature-major ([feature, token]) except v, which
is produced token-major so attn@v needs no transposes. Head pairs are packed
into PE row groups (rows 0-63 / 64-127) for the K=64 matmuls; each PSUM
"pair" tile is 2 banks wide (even head in columns 0-511, odd in 512-1023)
so one ACT exp covers both heads.
"""
import numpy as np
from contextlib import ExitStack

import ml_dtypes

B, S, D, H = 4, 2048, 1024, 16
HD = 64            # head dim
HPC = 8            # heads per core
F = HPC * HD       # 512 features per head-group
QT = 512           # q tile (free dim)
KT = 128           # k tile (partition dim)
NQI = S // QT      # 4
NKT = S // KT      # 16
NDK = D // 128     # 8 contraction tiles for projections
SCALE = HD ** -0.5

_CACHE = {}


def _build():
    import concourse.bacc as bacc
    import concourse.tile as tile
    import concourse.mybir as mybir

    f32 = mybir.dt.float32
    f32r = mybir.dt.float32r
    bf16 = mybir.dt.bfloat16
    EXP = mybir.ActivationFunctionType.Exp
    IDENT = mybir.ActivationFunctionType.Identity

    nc = bacc.Bacc("TRN2", target_bir_lowering=False, debug=False)
    xT = nc.dram_tensor("xT", [D, S], f32r, kind="ExternalInput").ap()
    w_sl = nc.dram_tensor("w_sl", [D, 3 * F], f32r, kind="ExternalInput").ap()
    wo_sl = nc.dram_tensor("wo_sl", [F, D], f32r, kind="ExternalInput").ap()
    bias_t = nc.dram_tensor("bias_t", [128, 8], f32, kind="ExternalInput").ap()
    mask2 = nc.dram_tensor("mask2", [128, 256], bf16, kind="ExternalInput").ap()
    out = nc.dram_tensor("out", [D, S], f32, kind="ExternalOutput").ap()

    with tile.TileContext(nc) as tc:
        with ExitStack() as ctx:
            # ---- persistent pools ----
            misc = ctx.enter_context(tc.tile_pool(name="misc", bufs=1))
            mask_sb = misc.tile([128, 256], bf16, name="mask_sb", tag="mask")
            nc.sync.dma_start(mask_sb[:], mask2)
            bias_sb = misc.tile([128, 8], f32, name="bias_sb", tag="bias")
            nc.sync.dma_start(bias_sb[:], bias_t)

            pqk = ctx.enter_context(tc.tile_pool(name="pqk", bufs=1))
            pv = ctx.enter_context(tc.tile_pool(name="pv", bufs=1))

            q_sb = [pqk.tile([128, S], bf16, name=f"q{g}", tag=f"q{g}")
                    for g in range(4)]
            k_sb = [pqk.tile([128, S], bf16, name=f"k{g}", tag=f"k{g}")
                    for g in range(4)]
            v_sb = [pv.tile([128, HPC * (HD + 1)], bf16, name=f"v{t}", tag=f"v{t}")
                    for t in range(NKT)]

            # ---- phase A: QKV projection (fp32r) ----
            with tc.tile_pool(name="xw", bufs=1) as xw, \
                 tc.tile_pool(name="psA", bufs=8, space="PSUM") as psA:
                x_t = []
                w_t = []
                for kk in range(NDK):
                    xt = xw.tile([128, S], f32r, name=f"x{kk}", tag=f"x{kk}")
                    nc.sync.dma_start(xt[:], xT[kk * 128:(kk + 1) * 128, :])
                    x_t.append(xt)
                    wt = xw.tile([128, 3 * F], f32r, name=f"w{kk}", tag=f"w{kk}")
                    nc.sync.dma_start(wt[:], w_sl[kk * 128:(kk + 1) * 128, :])
                    w_t.append(wt)

                # v first (token-major; x stationary, w moving) so attention
                # can begin as soon as early q/k token-groups drain.
                for t4 in range(NKT // 4):
                    pss = [psA.tile([128, F], f32, name=f"pv{t4}_{j}", tag="qkv")
                           for j in range(4)]
                    for kk in range(NDK):
                        for j in range(4):
                            tt = t4 * 4 + j
                            nc.tensor.matmul(
                                pss[j][:],
                                x_t[kk][:, tt * 128:(tt + 1) * 128],
                                w_t[kk][:, 2 * F:3 * F],
                                start=(kk == 0), stop=(kk == NDK - 1))
                    for j in range(4):
                        tt = t4 * 4 + j
                        vv = v_sb[tt].rearrange("p (h c) -> p h c", h=HPC)
                        pp = pss[j].rearrange("p (h c) -> p h c", h=HPC)
                        nc.vector.tensor_copy(vv[:, :, 0:HD], pp[:])
                        nc.vector.memset(vv[:, :, HD:HD + 1], 1.0)

                # q, k: feature-major [feat, tok]; w stationary, x moving.
                # tg inner so the tg=0 slices (needed by qi=0) drain first.
                for g in range(4):
                    for part, dest in ((0, q_sb), (1, k_sb)):
                        fcol = part * F + g * 128
                        pss = [psA.tile([128, QT], f32, name=f"pq{part}{g}{j}",
                                        tag="qkv") for j in range(NQI)]
                        for kk in range(NDK):
                            for tg in range(NQI):
                                nc.tensor.matmul(
                                    pss[tg][:],
                                    w_t[kk][:, fcol:fcol + 128],
                                    x_t[kk][:, tg * QT:(tg + 1) * QT],
                                    start=(kk == 0), stop=(kk == NDK - 1))
                        for tg in range(NQI):
                            nc.vector.tensor_copy(
                                dest[g][:, tg * QT:(tg + 1) * QT], pss[tg][:])

            # ---- phases B+C interleaved per q-tile ----
            patt = ctx.enter_context(tc.tile_pool(name="patt", bufs=1))
            pP = ctx.enter_context(tc.tile_pool(name="pP", bufs=4))
            pr = ctx.enter_context(tc.tile_pool(name="pr", bufs=2))
            prr = ctx.enter_context(tc.tile_pool(name="prr", bufs=2))
            pwo = ctx.enter_context(tc.tile_pool(name="pwo", bufs=1))
            pstg = ctx.enter_context(tc.tile_pool(name="pstg", bufs=3))

            wo_t = []
            for g in range(4):
                wt = pwo.tile([128, D], f32r, name=f"wo{g}", tag=f"wo{g}")
                nc.sync.dma_start(wt[:], wo_sl[g * 128:(g + 1) * 128, :])
                wo_t.append(wt)

            with tc.tile_pool(name="psB", bufs=2, space="PSUM") as psB:
                for qi in range(NQI):
                    nkt = 4 * qi + 4
                    qs = qi * QT
                    for pg in range(4):
                        he, ho = 2 * pg, 2 * pg + 1
                        C = HD + 1
                        ao = psB.tile([HD + 1, 2 * QT], f32,
                                      name=f"ao{pg}{qi}", tag="ao")
                        for kt in range(nkt):
                            d = kt - 4 * qi
                            n0 = 0 if d < 0 else 128 * d
                            kcol = slice(kt * 128, kt * 128 + 128)
                            sc = psB.tile([128, 2 * QT], f32,
                                          name=f"sc{pg}{qi}{kt}", tag="sc")
                            nc.tensor.matmul(
                                sc[:, n0:QT], k_sb[pg][0:64, kcol],
                                q_sb[pg][0:64, qs + n0:qs + QT],
                                start=True, stop=True)
                            nc.tensor.matmul(
                                sc[:, QT + n0:2 * QT], k_sb[pg][64:128, kcol],
                                q_sb[pg][64:128, qs + n0:qs + QT],
                                start=True, stop=True)
                            pt = pP.tile([128, 2 * QT], bf16,
                                         name=f"pt{pg}{qi}{kt}", tag="P")
                            sc3 = sc.rearrange("p (h c) -> p h c", h=2)
                            pt3 = pt.rearrange("p (h c) -> p h c", h=2)
                            nc.scalar.activation(pt3[:, :, n0:QT],
                                                 sc3[:, :, n0:QT],
                                                 EXP, scale=SCALE)
                            if d >= 0:
                                m3 = mask_sb.rearrange("p (h c) -> p h c", h=2)
                                nc.vector.tensor_mul(
                                    pt3[:, :, n0:n0 + 128],
                                    pt3[:, :, n0:n0 + 128], m3[:])
                            st = (kt == 0)
                            sp = (kt == nkt - 1)
                            nc.tensor.matmul(
                                ao[:, n0:QT], v_sb[kt][:, he * C:(he + 1) * C],
                                pt[:, n0:QT], start=st, stop=sp)
                            nc.tensor.matmul(
                                ao[:, QT + n0:2 * QT],
                                v_sb[kt][:, ho * C:(ho + 1) * C],
                                pt[:, QT + n0:2 * QT], start=st, stop=sp)

                        # normalize: r = 1/rowsum (row HD); gpsimd broadcasts
                        # partition 0 -> 0..63; DVE writes Q2/Q3 directly
                        am = patt.tile([128, QT], f32r,
                                       name=f"am{pg}{qi}", tag=f"am{pg}{qi}")
                        att_key = (pg, qi)
                        _ATT[att_key] = am
                        srow = prr.tile([1, 2 * QT], f32, name=f"sr{pg}{qi}",
                                        tag="sr")
                        nc.vector.tensor_copy(srow[:], ao[HD:HD + 1, :])
                        rr = prr.tile([1, 2 * QT], f32, name=f"rr{pg}{qi}",
                                      tag="rr")
                        nc.vector.reciprocal_approx_fast(rr[:], srow[:])
                        rb = pr.tile([HD, 2 * QT], f32, name=f"rb{pg}{qi}",
                                     tag="r")
                        nc.gpsimd.partition_broadcast(rb[:], rr[:], channels=HD)
                        nc.vector.tensor_mul(am[0:64, :], ao[0:HD, 0:QT],
                                             rb[:, 0:QT])
                        nc.vector.tensor_mul(am[64:128, :], ao[0:HD, QT:2 * QT],
                                             rb[:, QT:2 * QT])

                    # ---- phase C for this q-tile (shares the sc banks) ----
                    for dt in range(8):
                        dcol = slice(dt * 128, dt * 128 + 128)
                        ps = psB.tile([128, 2 * QT], f32,
                                      name=f"op{dt}{qi}", tag="sc")
                        for pg in range(4):
                            am = _ATT[(pg, qi)]
                            nc.tensor.matmul(
                                ps[:, 0:QT], wo_t[pg][0:64, dcol], am[0:64, :],
                                start=(pg == 0), stop=(pg == 3))
                            nc.tensor.matmul(
                                ps[:, QT:2 * QT], wo_t[pg][64:128, dcol],
                                am[64:128, :],
                                start=(pg == 0), stop=(pg == 3))
                        s1 = pstg.tile([128, QT], f32, name=f"s1{dt}{qi}",
                                       tag="s1")
                        nc.scalar.activation(s1[:], ps[:, 0:QT], IDENT,
                                             bias=bias_sb[:, dt:dt + 1])
                        s2 = pstg.tile([128, QT], f32, name=f"s2{dt}{qi}",
                                       tag="s2")
                        nc.vector.tensor_add(s2[:], ps[:, QT:2 * QT], s1[:])
                        nc.sync.dma_start(
                            out[dt * 128:(dt + 1) * 128,
                                qi * QT:(qi + 1) * QT], s2[:])

    nc.compile()
    return nc


_ATT = {}


def _get_nc():
    if "nc" not in _CACHE:
        _CACHE["nc"] = _build()
    return _CACHE["nc"]


def _prep_inputs(x, w_qkv, w_out, b_out):
    """Build the 8 per-core input maps."""
    x = np.asarray(x, dtype=np.float32)
    w_qkv = np.asarray(w_qkv, dtype=np.float32)
    w_out = np.asarray(w_out, dtype=np.float32)
    b_out = np.asarray(b_out, dtype=np.float32)

    tri = np.triu(np.ones((128, 128), dtype=np.float32))
    mask2 = np.tile(tri, (1, 2)).astype(ml_dtypes.bfloat16)
    zeros_bias = np.zeros((128, 8), dtype=np.float32)
    bias_t = np.ascontiguousarray(b_out.reshape(8, 128).T)

    in_maps = []
    for c in range(8):
        b, hg = c // 2, c % 2
        cols = hg * F
        w_cat = np.concatenate([
            w_qkv[:, cols:cols + F],
            w_qkv[:, D + cols:D + cols + F],
            w_qkv[:, 2 * D + cols:2 * D + cols + F],
        ], axis=1)
        in_maps.append({
            "xT": np.ascontiguousarray(x[b].T),
            "w_sl": np.ascontiguousarray(w_cat),
            "wo_sl": np.ascontiguousarray(w_out[cols:cols + F, :]),
            "bias_t": bias_t if hg == 0 else zeros_bias,
            "mask2": mask2,
        })
    return in_maps


def _run(inputs, trace=False):
    from concourse.bass_utils import run_bass_kernel_spmd

    nc = _get_nc()
    in_maps = _prep_inputs(**inputs)
    res = run_bass_kernel_spmd(nc, in_maps, core_ids=list(range(8)),
                               trace=trace)
    outs = []
    for b in range(B):
        o = res.results[2 * b]["out"] + res.results[2 * b + 1]["out"]
        outs.append(o.T)
    full = np.stack(outs).astype(np.float32)
    return full, res


def kernel(x, w_qkv, w_out, b_out):
    full, _ = _run({"x": x, "w_qkv": w_qkv, "w_out": w_out, "b_out": b_out})
    return full


# revision 10
# speedup vs baseline: 1.3206x; 1.0071x over previous
"""Causal multi-head attention (B=4, S=2048, D=1024, H=16) on 8 NeuronCores.

Sharding: core c = (batch b = c//2, head-group hg = c%2). Each core computes
8 heads of one batch: QKV projection (fp32r matmuls), causal flash-style
attention (bf16 matmuls, exp-without-max softmax with a ones-column
denominator), and a row-parallel out-projection partial. Host sums the two
head-group partials per batch and transposes.

Layouts are feature-major ([feature, token]) except v (token-major) so
attn@v needs no transposes. Head pairs are packed into PE row groups
(rows 0-63 / 64-127) for the K=64 matmuls; PSUM tiles are 2 banks wide
(even head in columns 0-511, odd in 512-1023) so one ACT exp covers both
heads. Emission interleaves QKV feature-groups, attention blocks, and
out-proj blocks so the single shared PSUM pool pipelines across phases.
"""
import numpy as np
from contextlib import ExitStack

import ml_dtypes

B, S, D, H = 4, 2048, 1024, 16
HD = 64            # head dim
HPC = 8            # heads per core
F = HPC * HD       # 512 features per head-group
QT = 512           # q tile (free dim)
NQI = S // QT      # 4
NKT = S // 128     # 16
NDK = D // 128     # 8 contraction tiles for projections
SCALE = HD ** -0.5

_CACHE = {}


def _build():
    import concourse.bacc as bacc
    import concourse.tile as tile
    import concourse.mybir as mybir

    f32 = mybir.dt.float32
    f32r = mybir.dt.float32r
    bf16 = mybir.dt.bfloat16
    EXP = mybir.ActivationFunctionType.Exp
    IDENT = mybir.ActivationFunctionType.Identity

    nc = bacc.Bacc("TRN2", target_bir_lowering=False, debug=False)
    xT = nc.dram_tensor("xT", [D, S], f32r, kind="ExternalInput").ap()
    w_sl = nc.dram_tensor("w_sl", [D, 3 * F], f32r, kind="ExternalInput").ap()
    wo_sl = nc.dram_tensor("wo_sl", [F, D], f32, kind="ExternalInput").ap()
    bias_t = nc.dram_tensor("bias_t", [128, 8], f32, kind="ExternalInput").ap()
    mask2 = nc.dram_tensor("mask2", [128, 256], bf16, kind="ExternalInput").ap()
    out = nc.dram_tensor("out", [D, S], f32, kind="ExternalOutput").ap()

    with tile.TileContext(nc) as tc:
        with ExitStack() as ctx:
            # ---- SBUF pools that must not overlap the x/w region ----
            misc = ctx.enter_context(tc.tile_pool(name="misc", bufs=1))
            mask_sb = misc.tile([128, 256], bf16, name="mask_sb", tag="mask")
            nc.sync.dma_start(mask_sb[:], mask2)
            bias_sb = misc.tile([128, 8], f32, name="bias_sb", tag="bias")
            nc.sync.dma_start(bias_sb[:], bias_t)

            pqk = ctx.enter_context(tc.tile_pool(name="pqk", bufs=1))
            pv = ctx.enter_context(tc.tile_pool(name="pv", bufs=1))
            patt = ctx.enter_context(tc.tile_pool(name="patt", bufs=8))
            pP = ctx.enter_context(tc.tile_pool(name="pP", bufs=2))
            pr = ctx.enter_context(tc.tile_pool(name="pr", bufs=1))
            prr = ctx.enter_context(tc.tile_pool(name="prr", bufs=2))
            pwo = ctx.enter_context(tc.tile_pool(name="pwo", bufs=1))

            wo_t = []
            for g in range(4):
                wt = pwo.tile([128, D], bf16, name=f"wo{g}", tag=f"wo{g}")
                # gpsimd DMA casts f32 -> bf16 in flight
                nc.gpsimd.dma_start(wt[:], wo_sl[g * 128:(g + 1) * 128, :])
                wo_t.append(wt)

            q_sb = [pqk.tile([128, S], bf16, name=f"q{g}", tag=f"q{g}")
                    for g in range(4)]
            k_sb = [pqk.tile([128, S], bf16, name=f"k{g}", tag=f"k{g}")
                    for g in range(4)]
            v_sb = [pv.tile([128, HPC * (HD + 1)], bf16, name=f"v{t}",
                            tag=f"v{t}") for t in range(NKT)]

            psum = ctx.enter_context(
                tc.tile_pool(name="psum", bufs=2, space="PSUM"))

            att_m = {}

            def attn_block(pg, qi):
                """Scores + exp + attn@v + normalize for head pair pg,
                q-range [qi*QT, (qi+1)*QT)."""
                nkt = 4 * qi + 4
                qs = qi * QT
                he, ho = 2 * pg, 2 * pg + 1
                C = HD + 1
                ao = psum.tile([HD + 1, 2 * QT], f32,
                               name=f"ao{pg}{qi}", tag="ao")
                for kt in range(nkt):
                    d = kt - 4 * qi
                    n0 = 0 if d < 0 else 128 * d
                    kcol = slice(kt * 128, kt * 128 + 128)
                    sc = psum.tile([128, 2 * QT], f32,
                                   name=f"sc{pg}{qi}{kt}", tag="big")
                    nc.tensor.matmul(
                        sc[:, n0:QT], k_sb[pg][0:64, kcol],
                        q_sb[pg][0:64, qs + n0:qs + QT],
                        start=True, stop=True)
                    nc.tensor.matmul(
                        sc[:, QT + n0:2 * QT], k_sb[pg][64:128, kcol],
                        q_sb[pg][64:128, qs + n0:qs + QT],
                        start=True, stop=True)
                    pt = pP.tile([128, 2 * QT], bf16,
                                 name=f"pt{pg}{qi}{kt}", tag="P")
                    sc3 = sc.rearrange("p (h c) -> p h c", h=2)
                    pt3 = pt.rearrange("p (h c) -> p h c", h=2)
                    nc.scalar.activation(pt3[:, :, n0:QT], sc3[:, :, n0:QT],
                                         EXP, scale=SCALE)
                    if d >= 0:
                        m3 = mask_sb.rearrange("p (h c) -> p h c", h=2)
                        nc.vector.tensor_mul(pt3[:, :, n0:n0 + 128],
                                             pt3[:, :, n0:n0 + 128], m3[:])
                    st = (kt == 0)
                    sp = (kt == nkt - 1)
                    nc.tensor.matmul(
                        ao[:, n0:QT], v_sb[kt][:, he * C:(he + 1) * C],
                        pt[:, n0:QT], start=st, stop=sp)
                    nc.tensor.matmul(
                        ao[:, QT + n0:2 * QT],
                        v_sb[kt][:, ho * C:(ho + 1) * C],
                        pt[:, QT + n0:2 * QT], start=st, stop=sp)

                # normalize: 1/rowsum (row HD) via fast recip + gpsimd bcast
                am = patt.tile([128, QT], bf16, name=f"am{pg}{qi}", tag="am")
                att_m[(pg, qi)] = am
                srow = prr.tile([1, 2 * QT], f32, name=f"sr{pg}{qi}", tag="sr")
                nc.vector.tensor_copy(srow[:], ao[HD:HD + 1, :])
                rr = prr.tile([1, 2 * QT], f32, name=f"rr{pg}{qi}", tag="rr")
                nc.vector.reciprocal_approx_fast(rr[:], srow[:])
                rb = pr.tile([HD, 2 * QT], f32, name=f"rb{pg}{qi}", tag="r")
                nc.gpsimd.partition_broadcast(rb[:], rr[:], channels=HD)
                nc.vector.tensor_mul(am[0:64, :], ao[0:HD, 0:QT], rb[:, 0:QT])
                nc.vector.tensor_mul(am[64:128, :], ao[0:HD, QT:2 * QT],
                                     rb[:, QT:2 * QT])

            with tc.tile_pool(name="xw", bufs=1) as xw:
                x_t = []
                wv_t = []
                wqk_t = []
                for kk in range(NDK):
                    r0 = slice(kk * 128, (kk + 1) * 128)
                    xt = xw.tile([128, S], f32r, name=f"x{kk}", tag=f"x{kk}")
                    nc.sync.dma_start(xt[:], xT[r0, :])
                    x_t.append(xt)
                    wv = xw.tile([128, F], f32r, name=f"wv{kk}", tag=f"wv{kk}")
                    nc.sync.dma_start(wv[:], w_sl[r0, 2 * F:3 * F])
                    wv_t.append(wv)
                for kk in range(NDK):
                    r0 = slice(kk * 128, (kk + 1) * 128)
                    wq = xw.tile([128, 2 * F], f32r, name=f"wq{kk}",
                                 tag=f"wq{kk}")
                    nc.sync.dma_start(wq[:], w_sl[r0, 0:2 * F])
                    wqk_t.append(wq)

                # ---- v projection (token-major), tt pairs per psum tile ----
                for t2 in range(NKT // 2):
                    ps = psum.tile([128, 2 * QT], f32, name=f"pv{t2}",
                                   tag="big")
                    for kk in range(NDK):
                        for j in range(2):
                            tt = 2 * t2 + j
                            nc.tensor.matmul(
                                ps[:, j * F:j * F + F],
                                x_t[kk][:, tt * 128:(tt + 1) * 128],
                                wv_t[kk][:],
                                start=(kk == 0), stop=(kk == NDK - 1))
                    for j in range(2):
                        tt = 2 * t2 + j
                        vv = v_sb[tt].rearrange("p (h c) -> p h c", h=HPC)
                        pp = ps[:, j * F:j * F + F].rearrange(
                            "p (h c) -> p h c", h=HPC)
                        nc.vector.tensor_copy(vv[:, :, 0:HD], pp[:])
                        nc.vector.memset(vv[:, :, HD:HD + 1], 1.0)

                # ---- q/k projection per feature group, then attention qi=0
                for g in range(4):
                    for part, dest in ((0, q_sb), (1, k_sb)):
                        fcol = part * F + g * 128
                        for th in range(2):  # tg pairs (0,1) and (2,3)
                            ps = psum.tile([128, 2 * QT], f32,
                                           name=f"pq{part}{g}{th}", tag="big")
                            for kk in range(NDK):
                                for j in range(2):
                                    tg = 2 * th + j
                                    nc.tensor.matmul(
                                        ps[:, j * QT:(j + 1) * QT],
                                        wqk_t[kk][:, fcol:fcol + 128],
                                        x_t[kk][:, tg * QT:(tg + 1) * QT],
                                        start=(kk == 0), stop=(kk == NDK - 1))
                            nc.vector.tensor_copy(
                                dest[g][:, th * 2 * QT:(th + 1) * 2 * QT],
                                ps[:])
                    attn_block(g, 0)

            # ---- remaining attention interleaved with out-proj; staging
            # pool reuses the released x/w space ----
            pstg = ctx.enter_context(tc.tile_pool(name="pstg", bufs=2))

            def out_block(qi):
                for dt in range(8):
                    dcol = slice(dt * 128, dt * 128 + 128)
                    ps = psum.tile([128, 2 * QT], f32,
                                   name=f"op{dt}{qi}", tag="big")
                    for pg in range(4):
                        am = att_m[(pg, qi)]
                        nc.tensor.matmul(
                            ps[:, 0:QT], wo_t[pg][0:64, dcol],
                            am[0:64, :],
                            start=(pg == 0), stop=(pg == 3))
                        nc.tensor.matmul(
                            ps[:, QT:2 * QT], wo_t[pg][64:128, dcol],
                            am[64:128, :],
                            start=(pg == 0), stop=(pg == 3))
                    s1 = pstg.tile([128, QT], f32, name=f"s1{dt}{qi}",
                                   tag="s1")
                    nc.scalar.activation(s1[:], ps[:, 0:QT], IDENT,
                                         bias=bias_sb[:, dt:dt + 1])
                    s2 = pstg.tile([128, QT], f32, name=f"s2{dt}{qi}",
                                   tag="s2")
                    nc.vector.tensor_add(s2[:], ps[:, QT:2 * QT], s1[:])
                    nc.sync.dma_start(
                        out[dt * 128:(dt + 1) * 128,
                            qi * QT:(qi + 1) * QT], s2[:])

            for qi in range(1, NQI):
                out_block(qi - 1)
                for pg in range(4):
                    attn_block(pg, qi)
            out_block(NQI - 1)

    nc.compile()
    return nc


def _get_nc():
    if "nc" not in _CACHE:
        _CACHE["nc"] = _build()
    return _CACHE["nc"]


def _prep_inputs(x, w_qkv, w_out, b_out):
    """Build the 8 per-core input maps."""
    x = np.asarray(x, dtype=np.float32)
    w_qkv = np.asarray(w_qkv, dtype=np.float32)
    w_out = np.asarray(w_out, dtype=np.float32)
    b_out = np.asarray(b_out, dtype=np.float32)

    tri = np.triu(np.ones((128, 128), dtype=np.float32))
    mask2 = np.tile(tri, (1, 2)).astype(ml_dtypes.bfloat16)
    zeros_bias = np.zeros((128, 8), dtype=np.float32)
    bias_t = np.ascontiguousarray(b_out.reshape(8, 128).T)

    in_maps = []
    for c in range(8):
        b, hg = c // 2, c % 2
        cols = hg * F
        w_cat = np.concatenate([
            w_qkv[:, cols:cols + F],
            w_qkv[:, D + cols:D + cols + F],
            w_qkv[:, 2 * D + cols:2 * D + cols + F],
        ], axis=1)
        in_maps.append({
            "xT": np.ascontiguousarray(x[b].T),
            "w_sl": np.ascontiguousarray(w_cat),
            "wo_sl": np.ascontiguousarray(w_out[cols:cols + F, :]),
            "bias_t": bias_t if hg == 0 else zeros_bias,
            "mask2": mask2,
        })
    return in_maps


def _run(inputs, trace=False):
    from concourse.bass_utils import run_bass_kernel_spmd

    nc = _get_nc()
    in_maps = _prep_inputs(**inputs)
    res = run_bass_kernel_spmd(nc, in_maps, core_ids=list(range(8)),
                               trace=trace)
    outs = []
    for b in range(B):
        o = res.results[2 * b]["out"] + res.results[2 * b + 1]["out"]
        outs.append(o.T)
    full = np.stack(outs).astype(np.float32)
    return full, res


def kernel(x, w_qkv, w_out, b_out):
    full, _ = _run({"x": x, "w_qkv": w_qkv, "w_out": w_out, "b_out": b_out})
    return full


# revision 11
# speedup vs baseline: 1.3843x; 1.0482x over previous
"""Causal multi-head attention (B=4, S=2048, D=1024, H=16) on 8 NeuronCores.

Sharding: core c = (batch b = c//2, head-group hg = c%2). Each core computes
8 heads of one batch: QKV projection (fp32r matmuls), causal flash-style
attention (bf16 matmuls, exp-without-max softmax with a ones-column
denominator), and a row-parallel out-projection partial. Host sums the two
head-group partials per batch and transposes.

Layouts are feature-major ([feature, token]) except v (token-major) so
attn@v needs no transposes. Head pairs are packed into PE row groups
(rows 0-63 / 64-127) for the K=64 matmuls; PSUM tiles are 2 banks wide
(even head in columns 0-511, odd in 512-1023) so one ACT exp covers both
heads. Emission interleaves QKV feature-groups, attention blocks, and
out-proj blocks so the single shared PSUM pool pipelines across phases.
"""
import numpy as np
from contextlib import ExitStack

import ml_dtypes

B, S, D, H = 4, 2048, 1024, 16
HD = 64            # head dim
HPC = 8            # heads per core
F = HPC * HD       # 512 features per head-group
QT = 512           # q tile (free dim)
NQI = S // QT      # 4
NKT = S // 128     # 16
NDK = D // 128     # 8 contraction tiles for projections
SCALE = HD ** -0.5

_CACHE = {}


def _build():
    import concourse.bacc as bacc
    import concourse.tile as tile
    import concourse.mybir as mybir

    f32 = mybir.dt.float32
    f32r = mybir.dt.float32r
    bf16 = mybir.dt.bfloat16
    EXP = mybir.ActivationFunctionType.Exp
    IDENT = mybir.ActivationFunctionType.Identity

    nc = bacc.Bacc("TRN2", target_bir_lowering=False, debug=False)
    xT = nc.dram_tensor("xT", [D, S], f32r, kind="ExternalInput").ap()
    w_sl = nc.dram_tensor("w_sl", [D, 3 * F], f32r, kind="ExternalInput").ap()
    wo_sl = nc.dram_tensor("wo_sl", [F, D], f32, kind="ExternalInput").ap()
    bias_t = nc.dram_tensor("bias_t", [128, 8], f32, kind="ExternalInput").ap()
    mask2 = nc.dram_tensor("mask2", [128, 256], bf16, kind="ExternalInput").ap()
    out = nc.dram_tensor("out", [D, S], f32, kind="ExternalOutput").ap()

    with tile.TileContext(nc) as tc:
        with ExitStack() as ctx:
            # ---- SBUF pools that must not overlap the x/w region ----
            misc = ctx.enter_context(tc.tile_pool(name="misc", bufs=1))
            mask_sb = misc.tile([128, 256], bf16, name="mask_sb", tag="mask")
            nc.sync.dma_start(mask_sb[:], mask2)
            bias_sb = misc.tile([128, 8], f32, name="bias_sb", tag="bias")
            nc.sync.dma_start(bias_sb[:], bias_t)

            pqk = ctx.enter_context(tc.tile_pool(name="pqk", bufs=1))
            pv = ctx.enter_context(tc.tile_pool(name="pv", bufs=1))
            patt = ctx.enter_context(tc.tile_pool(name="patt", bufs=16))
            pP = ctx.enter_context(tc.tile_pool(name="pP", bufs=2))
            pr = ctx.enter_context(tc.tile_pool(name="pr", bufs=1))
            prr = ctx.enter_context(tc.tile_pool(name="prr", bufs=1))
            pwo = ctx.enter_context(tc.tile_pool(name="pwo", bufs=1))

            wo_t = []
            for g in range(4):
                wt = pwo.tile([128, D], bf16, name=f"wo{g}", tag=f"wo{g}")
                # gpsimd DMA casts f32 -> bf16 in flight
                nc.gpsimd.dma_start(wt[:], wo_sl[g * 128:(g + 1) * 128, :])
                wo_t.append(wt)

            q_sb = [pqk.tile([128, S], bf16, name=f"q{g}", tag=f"q{g}")
                    for g in range(4)]
            k_sb = [pqk.tile([128, S], bf16, name=f"k{g}", tag=f"k{g}")
                    for g in range(4)]
            v_sb = [pv.tile([128, HPC * (HD + 1)], bf16, name=f"v{t}",
                            tag=f"v{t}") for t in range(NKT)]

            psum = ctx.enter_context(
                tc.tile_pool(name="psum", bufs=2, space="PSUM"))

            att_m = {}

            def attn_block(pg, qi):
                """Scores + exp + attn@v + normalize for head pair pg,
                q-range [qi*QT, (qi+1)*QT)."""
                nkt = 4 * qi + 4
                qs = qi * QT
                he, ho = 2 * pg, 2 * pg + 1
                C = HD + 1
                ao = psum.tile([HD + 1, 2 * QT], f32,
                               name=f"ao{pg}{qi}", tag="ao")
                for kt in range(nkt):
                    d = kt - 4 * qi
                    n0 = 0 if d < 0 else 128 * d
                    kcol = slice(kt * 128, kt * 128 + 128)
                    sc = psum.tile([128, 2 * QT], f32,
                                   name=f"sc{pg}{qi}{kt}", tag="big")
                    nc.tensor.matmul(
                        sc[:, n0:QT], k_sb[pg][0:64, kcol],
                        q_sb[pg][0:64, qs + n0:qs + QT],
                        start=True, stop=True)
                    nc.tensor.matmul(
                        sc[:, QT + n0:2 * QT], k_sb[pg][64:128, kcol],
                        q_sb[pg][64:128, qs + n0:qs + QT],
                        start=True, stop=True)
                    pt = pP.tile([128, 2 * QT], bf16,
                                 name=f"pt{pg}{qi}{kt}", tag="P")
                    sc3 = sc.rearrange("p (h c) -> p h c", h=2)
                    pt3 = pt.rearrange("p (h c) -> p h c", h=2)
                    nc.scalar.activation(pt3[:, :, n0:QT], sc3[:, :, n0:QT],
                                         EXP, scale=SCALE)
                    if d >= 0:
                        m3 = mask_sb.rearrange("p (h c) -> p h c", h=2)
                        nc.vector.tensor_mul(pt3[:, :, n0:n0 + 128],
                                             pt3[:, :, n0:n0 + 128], m3[:])
                    st = (kt == 0)
                    sp = (kt == nkt - 1)
                    nc.tensor.matmul(
                        ao[:, n0:QT], v_sb[kt][:, he * C:(he + 1) * C],
                        pt[:, n0:QT], start=st, stop=sp)
                    nc.tensor.matmul(
                        ao[:, QT + n0:2 * QT],
                        v_sb[kt][:, ho * C:(ho + 1) * C],
                        pt[:, QT + n0:2 * QT], start=st, stop=sp)

                # normalize: 1/rowsum (row HD) via fast recip + gpsimd bcast
                am = patt.tile([128, QT], bf16, name=f"am{pg}{qi}", tag="am")
                att_m[(pg, qi)] = am
                srow = prr.tile([1, 2 * QT], f32, name=f"sr{pg}{qi}", tag="sr")
                nc.vector.tensor_copy(srow[:], ao[HD:HD + 1, :])
                rr = prr.tile([1, 2 * QT], f32, name=f"rr{pg}{qi}", tag="rr")
                nc.vector.reciprocal_approx_fast(rr[:], srow[:])
                rb = pr.tile([HD, 2 * QT], f32, name=f"rb{pg}{qi}", tag="r")
                nc.gpsimd.partition_broadcast(rb[:], rr[:], channels=HD)
                nc.vector.tensor_mul(am[0:64, :], ao[0:HD, 0:QT], rb[:, 0:QT])
                nc.vector.tensor_mul(am[64:128, :], ao[0:HD, QT:2 * QT],
                                     rb[:, QT:2 * QT])

            with tc.tile_pool(name="xw", bufs=1) as xw:
                x_t = []
                wv_t = []
                wqk_t = []
                for kk in range(NDK):
                    r0 = slice(kk * 128, (kk + 1) * 128)
                    xt = xw.tile([128, S], f32r, name=f"x{kk}", tag=f"x{kk}")
                    nc.sync.dma_start(xt[:], xT[r0, :])
                    x_t.append(xt)
                    wv = xw.tile([128, F], f32r, name=f"wv{kk}", tag=f"wv{kk}")
                    nc.sync.dma_start(wv[:], w_sl[r0, 2 * F:3 * F])
                    wv_t.append(wv)
                for kk in range(NDK):
                    r0 = slice(kk * 128, (kk + 1) * 128)
                    wq = xw.tile([128, 2 * F], f32r, name=f"wq{kk}",
                                 tag=f"wq{kk}")
                    nc.sync.dma_start(wq[:], w_sl[r0, 0:2 * F])
                    wqk_t.append(wq)

                # ---- v projection (token-major), tt pairs per psum tile ----
                for t2 in range(NKT // 2):
                    ps = psum.tile([128, 2 * QT], f32, name=f"pv{t2}",
                                   tag="big")
                    for kk in range(NDK):
                        for j in range(2):
                            tt = 2 * t2 + j
                            nc.tensor.matmul(
                                ps[:, j * F:j * F + F],
                                x_t[kk][:, tt * 128:(tt + 1) * 128],
                                wv_t[kk][:],
                                start=(kk == 0), stop=(kk == NDK - 1))
                    for j in range(2):
                        tt = 2 * t2 + j
                        vv = v_sb[tt].rearrange("p (h c) -> p h c", h=HPC)
                        pp = ps[:, j * F:j * F + F].rearrange(
                            "p (h c) -> p h c", h=HPC)
                        nc.vector.tensor_copy(vv[:, :, 0:HD], pp[:])
                        nc.vector.memset(vv[:, :, HD:HD + 1], 1.0)

                # ---- q/k projection per feature group, then attention qi=0
                for g in range(4):
                    for part, dest in ((0, q_sb), (1, k_sb)):
                        fcol = part * F + g * 128
                        for th in range(2):  # tg pairs (0,1) and (2,3)
                            ps = psum.tile([128, 2 * QT], f32,
                                           name=f"pq{part}{g}{th}", tag="big")
                            for kk in range(NDK):
                                for j in range(2):
                                    tg = 2 * th + j
                                    nc.tensor.matmul(
                                        ps[:, j * QT:(j + 1) * QT],
                                        wqk_t[kk][:, fcol:fcol + 128],
                                        x_t[kk][:, tg * QT:(tg + 1) * QT],
                                        start=(kk == 0), stop=(kk == NDK - 1))
                            nc.vector.tensor_copy(
                                dest[g][:, th * 2 * QT:(th + 1) * 2 * QT],
                                ps[:])
                    for qi in range(NQI):
                        attn_block(g, qi)

            # ---- remaining attention interleaved with out-proj; staging
            # pool reuses the released x/w space ----
            pstg = ctx.enter_context(tc.tile_pool(name="pstg", bufs=2))

            def out_block(qi):
                for dt in range(8):
                    dcol = slice(dt * 128, dt * 128 + 128)
                    ps = psum.tile([128, 2 * QT], f32,
                                   name=f"op{dt}{qi}", tag="big")
                    for pg in range(4):
                        am = att_m[(pg, qi)]
                        nc.tensor.matmul(
                            ps[:, 0:QT], wo_t[pg][0:64, dcol],
                            am[0:64, :],
                            start=(pg == 0), stop=(pg == 3))
                        nc.tensor.matmul(
                            ps[:, QT:2 * QT], wo_t[pg][64:128, dcol],
                            am[64:128, :],
                            start=(pg == 0), stop=(pg == 3))
                    s1 = pstg.tile([128, QT], f32, name=f"s1{dt}{qi}",
                                   tag="s1")
                    nc.vector.tensor_scalar_add(s1[:], ps[:, 0:QT],
                                                bias_sb[:, dt:dt + 1])
                    s2 = pstg.tile([128, QT], f32, name=f"s2{dt}{qi}",
                                   tag="s2")
                    nc.vector.tensor_add(s2[:], ps[:, QT:2 * QT], s1[:])
                    nc.sync.dma_start(
                        out[dt * 128:(dt + 1) * 128,
                            qi * QT:(qi + 1) * QT], s2[:])

            for qi in range(NQI):
                out_block(qi)

    nc.compile()
    return nc


def _get_nc():
    if "nc" not in _CACHE:
        _CACHE["nc"] = _build()
    return _CACHE["nc"]


def _prep_inputs(x, w_qkv, w_out, b_out):
    """Build the 8 per-core input maps."""
    x = np.asarray(x, dtype=np.float32)
    w_qkv = np.asarray(w_qkv, dtype=np.float32)
    w_out = np.asarray(w_out, dtype=np.float32)
    b_out = np.asarray(b_out, dtype=np.float32)

    tri = np.triu(np.ones((128, 128), dtype=np.float32))
    mask2 = np.tile(tri, (1, 2)).astype(ml_dtypes.bfloat16)
    zeros_bias = np.zeros((128, 8), dtype=np.float32)
    bias_t = np.ascontiguousarray(b_out.reshape(8, 128).T)

    in_maps = []
    for c in range(8):
        b, hg = c // 2, c % 2
        cols = hg * F
        w_cat = np.concatenate([
            w_qkv[:, cols:cols + F],
            w_qkv[:, D + cols:D + cols + F],
            w_qkv[:, 2 * D + cols:2 * D + cols + F],
        ], axis=1)
        in_maps.append({
            "xT": np.ascontiguousarray(x[b].T),
            "w_sl": np.ascontiguousarray(w_cat),
            "wo_sl": np.ascontiguousarray(w_out[cols:cols + F, :]),
            "bias_t": bias_t if hg == 0 else zeros_bias,
            "mask2": mask2,
        })
    return in_maps


def _run(inputs, trace=False):
    from concourse.bass_utils import run_bass_kernel_spmd

    nc = _get_nc()
    in_maps = _prep_inputs(**inputs)
    res = run_bass_kernel_spmd(nc, in_maps, core_ids=list(range(8)),
                               trace=trace)
    outs = []
    for b in range(B):
        o = res.results[2 * b]["out"] + res.results[2 * b + 1]["out"]
        outs.append(o.T)
    full = np.stack(outs).astype(np.float32)
    return full, res


def kernel(x, w_qkv, w_out, b_out):
    full, _ = _run({"x": x, "w_qkv": w_qkv, "w_out": w_out, "b_out": b_out})
    return full
